# revision 22
# baseline (speedup 1.0000x reference)
"""Trainium2 Bass kernel for nn_BattleModel (segment_reduce).

Self-contained: host-side sharding/layout prep + Bass program + SPMD run
on 8 NeuronCores. See inline notes for the layout algebra.

Algorithm (per core; segments sharded 2048/core so segment_sum is local):
- Per-unit MLP h=relu(x@W1+b1) runs on TensorE as K=26 matmuls over a
  feature-across-partition packed layout (bias enters as a 10th input
  feature equal to 1.0, so zero-padded columns contribute exactly 0).
- ScalarE/VectorE apply relu while evicting PSUM -> fp16 H[128, Upc]
  (left features on partitions 0:64, right on 64:128).
- Segment pooling: units are pre-placed (host-side) so that a 2-level
  pairwise add tree yields per-4-unit chunk sums in segment-sorted slot
  order; a handful of grouped tensor_reduce calls (one per distinct
  chunks-per-segment m) produce the pooled features.  Group sizes are
  padded to the max across cores so the program is SPMD-identical.
- Combine MLP (128->32->1) + sigmoid on TensorE/ScalarE; host inverts the
  segment sort when assembling the [16384] output.
"""

from contextlib import ExitStack

import numpy as np

import concourse.bacc as bacc
import concourse.bass as bass
import concourse.tile as tile
from concourse import mybir
from concourse.bass_utils import run_bass_kernel_spmd

N_UNITS = 524288
BATCH = 16384
N_CORES = 8
SEG_PER_CORE = BATCH // N_CORES  # 2048
CHUNK = 4
SB_COLS = 8192
KSUB = SB_COLS // CHUNK  # 2048
NFEAT = 9
F16 = 16

# eviction engine per lane q (0..7): True -> ScalarE (ACT), False -> VectorE
EVICT_ON_ACT = [True, True, True, True, True, False, False, False]
# tree split: leading columns of each level go to GPSIMD, rest to VectorE
T1_GPS = 2048  # of 4096
T2_GPS = 1024  # of 2048


# ---------------------------------------------------------------- host prep

def _side_counts(seg, core):
    lo = int(np.searchsorted(seg, SEG_PER_CORE * core, side="left"))
    hi = int(np.searchsorted(seg, SEG_PER_CORE * (core + 1), side="left"))
    counts = np.bincount(
        seg[lo:hi] - SEG_PER_CORE * core, minlength=SEG_PER_CORE
    ).astype(np.int64)
    return lo, counts


def _col_of_slot_unit(slot, k):
    return (slot // KSUB) * SB_COLS + (slot % KSUB) + k * KSUB


def _upload_pos(u):
    sb, rem = u // SB_COLS, u % SB_COLS
    beta, r = rem // 512, rem % 512
    wpar, q = beta // 8, beta % 8
    return sb * 1024 + wpar * 512 + r, q


def host_prep(left_feats, right_feats, left_seg, right_seg):
    left_feats = np.asarray(left_feats, dtype=np.float32)
    right_feats = np.asarray(right_feats, dtype=np.float32)
    left_seg = np.asarray(left_seg)
    right_seg = np.asarray(right_seg)

    cores = []
    for d in range(N_CORES):
        loL, cL = _side_counts(left_seg, d)
        loR, cR = _side_counts(right_seg, d)
        m = np.maximum(
            np.maximum((cL + CHUNK - 1) // CHUNK, (cR + CHUNK - 1) // CHUNK), 1
        )
        order = np.argsort(m, kind="stable")
        cores.append(
            dict(loL=loL, loR=loR, cL=cL, cR=cR, m=m, order=order,
                 m_sorted=m[order])
        )

    m_max = int(max(int(c["m_sorted"][-1]) for c in cores))
    N = np.zeros(m_max + 1, dtype=np.int64)
    for c in cores:
        N = np.maximum(N, np.bincount(c["m_sorted"], minlength=m_max + 1))
    seg_start = np.zeros(m_max + 2, dtype=np.int64)
    slot_start = np.zeros(m_max + 2, dtype=np.int64)
    for m in range(1, m_max + 1):
        seg_start[m + 1] = seg_start[m] + N[m]
        slot_start[m + 1] = slot_start[m] + N[m] * m
    poolw_real = int(seg_start[m_max + 1])
    m_tot = int(slot_start[m_max + 1])
    m_pad = -(-m_tot // KSUB) * KSUB
    upc = m_pad * CHUNK
    poolw = -(-poolw_real // 512) * 512
    groups = [
        (m, int(seg_start[m]), int(N[m]), int(slot_start[m]))
        for m in range(1, m_max + 1)
        if N[m] > 0
    ]

    for c in cores:
        m_sorted = c["m_sorted"]
        grp_pos = np.zeros(SEG_PER_CORE, dtype=np.int64)
        prev_m, pos = -1, 0
        for j in range(SEG_PER_CORE):
            mj = int(m_sorted[j])
            pos = pos + 1 if mj == prev_m else 0
            grp_pos[j] = pos
            prev_m = mj
        c["seg_col"] = seg_start[m_sorted] + grp_pos
        c["slot_base"] = slot_start[m_sorted] + grp_pos * m_sorted
        col2seg = np.full(poolw, -1, dtype=np.int64)
        col2seg[c["seg_col"]] = c["order"]
        c["col2seg"] = col2seg

        for side in ("L", "R"):
            feats = left_feats if side == "L" else right_feats
            lo = c["loL"] if side == "L" else c["loR"]
            cnt = c["cL"] if side == "L" else c["cR"]
            src_start = np.zeros(SEG_PER_CORE, dtype=np.int64)
            src_start[1:] = np.cumsum(cnt)[:-1]
            c_sorted = cnt[c["order"]]
            src_start_sorted = src_start[c["order"]]
            tot = int(c_sorted.sum())
            i_in_seg = np.arange(tot) - np.repeat(
                np.cumsum(c_sorted) - c_sorted, c_sorted
            )
            src_row = lo + np.repeat(src_start_sorted, c_sorted) + i_in_seg
            slot = np.repeat(c["slot_base"], c_sorted) + i_in_seg // CHUNK
            u = _col_of_slot_unit(slot, i_in_seg % CHUNK)
            R, q = _upload_pos(u)
            xh = np.zeros((upc // 8, 128), dtype=np.float16)
            for f in range(NFEAT):
                xh[R, 16 * q + f] = feats[src_row, f].astype(np.float16)
            xh[R, 16 * q + NFEAT] = 1.0
            c["xh" + side] = xh

    return dict(upc=upc, m_pad=m_pad, poolw=poolw, poolw_real=poolw_real,
                groups=groups, cores=cores, m_max=m_max)


def make_weight_arrays(W1, b1, Wc1, bc1, Wc2, bc2):
    W1plus = np.concatenate(
        [np.asarray(W1, np.float32), np.asarray(b1, np.float32)[None, :]], axis=0
    )
    w1s = np.zeros((128, 256), dtype=np.float16)
    for i in range(4):
        for p in range(2):
            for side in range(2):
                colblk = (2 * p + side) * 64
                rowoff = 32 * i + 16 * p
                w1s[rowoff : rowoff + 10, colblk : colblk + 64] = W1plus.astype(
                    np.float16
                )
    wpack = np.zeros((128, 35), dtype=np.float32)
    wpack[:, 0:32] = np.asarray(Wc1, np.float32)
    wpack[0:32, 32] = np.asarray(Wc2, np.float32)[:, 0]
    wpack[0:32, 33] = np.asarray(bc1, np.float32)
    wpack[0, 34] = np.asarray(bc2, np.float32)[0]
    return dict(w1s=w1s, wpack=wpack)


# ------------------------------------------------------------- bass program

def build_nc(upc, m_pad, poolw, groups):
    f16, f32 = mybir.dt.float16, mybir.dt.float32
    nc = bacc.Bacc()
    nrows = upc // 8
    n_waves = upc // 4096
    n_sb = upc // SB_COLS

    xh_dram = {
        s: nc.declare_dram_parameter("xh" + s, [nrows, 128], f16, isOutput=False)
        for s in ("L", "R")
    }
    w1s_dram = nc.declare_dram_parameter("w1s", [128, 256], f16, isOutput=False)
    # wpack[:, 0:32] = Wc1; wpack[0:32, 32] = Wc2; wpack[0:32, 33] = bc1;
    # wpack[0, 34] = bc2
    wpack_dram = nc.declare_dram_parameter("wpack", [128, 35], f32, isOutput=False)
    out_dram = nc.declare_dram_parameter("out", [1, poolw], f32, isOutput=True)

    relu = mybir.ActivationFunctionType.Relu
    sigmoid = mybir.ActivationFunctionType.Sigmoid
    add = mybir.AluOpType.add
    alu_max = mybir.AluOpType.max

    with tile.TileContext(nc) as tc, ExitStack() as ctx:
        consts = ctx.enter_context(tc.tile_pool(name="consts", bufs=1))
        xt_pool = ctx.enter_context(tc.tile_pool(name="xt", bufs=1))
        big = ctx.enter_context(tc.tile_pool(name="big", bufs=1))
        h16_pool = ctx.enter_context(tc.tile_pool(name="h16", bufs=4))
        t1_pool = ctx.enter_context(tc.tile_pool(name="t1", bufs=3))

        w1s = consts.tile([128, 256], f16)
        nc.sync.dma_start(w1s[:], w1s_dram[:])
        wpack = consts.tile([128, 35], f32)
        nc.sync.dma_start(wpack[:], wpack_dram[:])
        wc1 = wpack[:, 0:32]
        wc2 = wpack[0:32, 32:33]
        bc1 = wpack[0:32, 33:34]
        bc2 = wpack[0:1, 34:35]

        xt = {s: xt_pool.tile([128, nrows], f16, name="xt" + s) for s in ("L", "R")}
        n_ldc = 4
        ldrows = nrows // n_ldc
        assert ldrows % 16 == 0
        for cchunk in range(n_ldc):
            for s in ("L", "R"):
                r0 = cchunk * ldrows
                nc.sync.dma_start_transpose(
                    xt[s][:, r0 : r0 + ldrows], xh_dram[s][r0 : r0 + ldrows, :]
                )

        chunks = big.tile([128, m_pad], f16)
        pooledT = big.tile([128, poolw], f32)
        nc.gpsimd.memset(pooledT[:], 0.0)

        with tc.tile_pool(name="psum", bufs=8, space="PSUM") as psum_pool:
            for sb in range(n_sb):
                h16 = h16_pool.tile([128, SB_COLS], f16, tag="h16")
                for wpar in range(2):
                    wv = 2 * sb + wpar
                    for i in range(4):
                        for p in range(2):
                            q = 2 * i + p
                            pt = psum_pool.tile([128, 512], f32, tag="pt")
                            for side in range(2):
                                nc.tensor.matmul(
                                    pt[64 * side : 64 * side + 64, :],
                                    w1s[
                                        32 * i : 32 * i + 26,
                                        (2 * p + side) * 64
                                        : (2 * p + side + 1) * 64,
                                    ],
                                    xt["L" if side == 0 else "R"][
                                        32 * i : 32 * i + 26,
                                        512 * wv : 512 * wv + 512,
                                    ],
                                    start=True,
                                    stop=True,
                                    tile_position=(32 * i, 64 * side),
                                    skip_group_check=True,
                                )
                            col0 = 512 * (8 * wpar + q)
                            dst = h16[:, col0 : col0 + 512]
                            if EVICT_ON_ACT[q]:
                                nc.scalar.activation(dst, pt[:], relu)
                            else:
                                nc.vector.tensor_scalar(dst, pt[:], 0.0, None,
                                                        alu_max)
                # superblock fully evicted: 2-level add tree -> chunk sums
                half = SB_COLS // 2
                t1 = t1_pool.tile([128, half], f16, tag="t1")
                nc.gpsimd.tensor_tensor(
                    t1[:, :T1_GPS], h16[:, :T1_GPS],
                    h16[:, half : half + T1_GPS], add,
                )
                nc.vector.tensor_tensor(
                    t1[:, T1_GPS:], h16[:, T1_GPS:half],
                    h16[:, half + T1_GPS :], add,
                )
                cdst = chunks[:, sb * KSUB : (sb + 1) * KSUB]
                nc.gpsimd.tensor_tensor(
                    cdst[:, :T2_GPS], t1[:, :T2_GPS],
                    t1[:, KSUB : KSUB + T2_GPS], add,
                )
                nc.vector.tensor_tensor(
                    cdst[:, T2_GPS:], t1[:, T2_GPS:KSUB],
                    t1[:, KSUB + T2_GPS :], add,
                )

        for m, seg0, n, slot0 in groups:
            src = chunks[:, slot0 : slot0 + n * m]
            nc.vector.tensor_reduce(
                pooledT[:, seg0 : seg0 + n],
                src.rearrange("p (n m) -> p n m", m=m),
                axis=mybir.AxisListType.X,
                op=add,
            )

        hiddenT = big.tile([32, poolw], f32)
        outbuf = big.tile([1, poolw], f32)
        with tc.tile_pool(name="psum2", bufs=2, space="PSUM") as psum2_pool:
            for cchunk in range(poolw // 512):
                c0 = cchunk * 512
                ph = psum2_pool.tile([32, 512], f32, tag="ph")
                nc.tensor.matmul(
                    ph[:], wc1[:], pooledT[:, c0 : c0 + 512], start=True, stop=True
                )
                nc.scalar.activation(
                    hiddenT[:, c0 : c0 + 512], ph[:], relu, bias=bc1[:, 0:1]
                )
                pl = psum2_pool.tile([1, 512], f32, tag="pl")
                nc.tensor.matmul(
                    pl[:], wc2[:], hiddenT[:, c0 : c0 + 512], start=True, stop=True
                )
                nc.scalar.activation(
                    outbuf[:, c0 : c0 + 512], pl[:], sigmoid, bias=bc2[:, 0:1]
                )
        nc.sync.dma_start(out_dram[:], outbuf[:])

    nc.compile()
    return nc


# ------------------------------------------------------------------- driver

def kernel(**inputs):
    meta = host_prep(
        inputs["left_feats"], inputs["right_feats"],
        inputs["left_seg"], inputs["right_seg"],
    )
    wts = make_weight_arrays(
        inputs["W1"], inputs["b1"], inputs["Wc1"], inputs["bc1"],
        inputs["Wc2"], inputs["bc2"],
    )
    nc = build_nc(meta["upc"], meta["m_pad"], meta["poolw"], meta["groups"])
    in_maps = []
    for d in range(N_CORES):
        c = meta["cores"][d]
        in_maps.append(
            dict(xhL=c["xhL"], xhR=c["xhR"], w1s=wts["w1s"], wpack=wts["wpack"])
        )
    res = run_bass_kernel_spmd(nc, in_maps, core_ids=list(range(N_CORES)))
    global _last_results
    _last_results = res
    out = np.zeros(BATCH, dtype=np.float32)
    for d in range(N_CORES):
        col2seg = meta["cores"][d]["col2seg"]
        valid = col2seg >= 0
        dev = np.asarray(res.results[d]["out"]).reshape(-1)
        out[SEG_PER_CORE * d + col2seg[valid]] = dev[valid]
    return out
